# revision 1
# baseline (speedup 1.0000x reference)
"""Trainium2 Bass kernel for nn_Cross_Attention (B=2, C=128, HEADS=4, N=16^3).

Sharding: 8 cores = (batch b in {0,1}) x (query-quarter iq in {0..3}).
Each core runs the full q/k/v projections (cheap; needed because the L2
normalization is over the FULL spatial axis), then attends only for its own
1024 query positions and produces the final output columns for them.
The host rotates x[b] per core so the core's query chunk is at columns
0:1024 -- the device program is identical across cores (SPMD), and the
rotation does not change the per-row norms.

Self-contained: only imports concourse (on PYTHONPATH in this environment)
and numpy.
"""

import os
from contextlib import ExitStack

import numpy as np

import concourse.bass as bass
import concourse.bacc as bacc
import concourse.tile as tile
from concourse import mybir
from concourse import bass_utils

P = 128          # channels / partitions
N = 4096         # spatial positions (16*16*16)
HEADS = 4
D = 32           # head dim
IPC = 1024       # query positions per core
NCORES = 8
SCALE = 10.0
EPS2 = 1e-24     # eps^2 for F.normalize(eps=1e-12)

f32 = mybir.dt.float32
bf16 = mybir.dt.bfloat16
f32r = mybir.dt.float32r
AF = mybir.ActivationFunctionType

LAST_RESULTS = None  # test harness reads exec_time_ns from here


def _build_program():
    nc = bacc.Bacc("TRN2", target_bir_lowering=False, debug=False,
                   num_devices=NCORES)

    xb = nc.dram_tensor("xb", [P, N], f32, kind="ExternalInput").ap()
    cb = nc.dram_tensor("cb", [P, N], f32, kind="ExternalInput").ap()
    wqT = nc.dram_tensor("wqT", [P, P], f32, kind="ExternalInput").ap()
    wkT = nc.dram_tensor("wkT", [P, P], f32, kind="ExternalInput").ap()
    wvT = nc.dram_tensor("wvT", [P, P], f32, kind="ExternalInput").ap()
    woT = nc.dram_tensor("woT", [P, P], f32, kind="ExternalInput").ap()
    bq = nc.dram_tensor("bq", [P, 1], f32, kind="ExternalInput").ap()
    bk = nc.dram_tensor("bk", [P, 1], f32, kind="ExternalInput").ap()
    boe = nc.dram_tensor("boe", [P, 1], f32, kind="ExternalInput").ap()
    y = nc.dram_tensor("y", [P, IPC], f32, kind="ExternalOutput").ap()

    with tile.TileContext(nc) as tc:
        _emit(tc, xb, cb, wqT, wkT, wvT, woT, bq, bk, boe, y)
    nc.compile()
    return nc


def _emit(tc, xb_d, cb_d, wqT_d, wkT_d, wvT_d, woT_d, bq_d, bk_d, boe_d, y_d):
    nc = tc.nc
    with ExitStack() as ctx:
        const = ctx.enter_context(tc.tile_pool(name="const", bufs=1))
        big = ctx.enter_context(tc.tile_pool(name="big", bufs=1))
        stat = ctx.enter_context(tc.tile_pool(name="stat", bufs=1))

        # ---- PE warm-up: ~25 dependency-free matmuls on memset data keep
        # the PE busy from t=0 so the HAM clock-gate opens (1.2 -> 2.4 GHz)
        # before the real work arrives; later gaps are all < the ~3.4us
        # re-throttle window, so it stays warm.
        wm_w = const.tile([P, P], bf16)
        nc.vector.memset(wm_w[:], 0.5)
        wm_x = const.tile([P, 512], bf16)
        nc.vector.memset(wm_x[:], 0.25)
        with tc.tile_pool(name="psW", bufs=1, space="PSUM") as psW:
            wm_ps = psW.tile([P, 512], f32)
            for _ in range(32):
                nc.tensor.matmul(wm_ps[:], lhsT=wm_w[:], rhs=wm_x[:],
                                 start=True, stop=True, skip_group_check=True)

        # ---- input DMA
        xb = big.tile([P, N], f32)
        cb = big.tile([P, N], f32)
        nc.sync.dma_start(cb[:], cb_d)
        nc.sync.dma_start(xb[:], xb_d)
        wqT = const.tile([P, P], f32)
        wkT = const.tile([P, P], f32)
        wvT = const.tile([P, P], f32)
        woT = const.tile([P, P], f32)
        for t, d in ((wvT, wvT_d), (wqT, wqT_d), (wkT, wkT_d), (woT, woT_d)):
            nc.sync.dma_start(t[:], d)
        bq = const.tile([P, 1], f32)
        bk = const.tile([P, 1], f32)
        boe = const.tile([P, 1], f32)
        for t, d in ((bq, bq_d), (bk, bk_d), (boe, boe_d)):
            nc.sync.dma_start(t[:], d)

        # ---- constants
        ones_sp = const.tile([P, 32], f32)
        nc.vector.memset(ones_sp[:], 1.0)
        woT_bf = const.tile([P, P], bf16)
        nc.vector.tensor_copy(woT_bf[:], woT[:])
        wqT_bf = const.tile([P, P], bf16)
        nc.vector.tensor_copy(wqT_bf[:], wqT[:])
        wkT_bf = const.tile([P, P], bf16)
        nc.vector.tensor_copy(wkT_bf[:], wkT[:])
        wvT_bf = const.tile([P, P], bf16)
        nc.vector.tensor_copy(wvT_bf[:], wvT[:])
        xb_bf = big.tile([P, N], bf16)
        nc.vector.tensor_copy(xb_bf[:], xb[:])
        cb_bf = big.tile([P, N], bf16)
        nc.vector.tensor_copy(cb_bf[:], cb[:])

        # ---- vT projection: vT[n, c] = sum_c' cond[c', n] * WvT[c', c]
        # Stored augmented per jblk: 4 head-blocks of 33 cols (32 v-cols +
        # a ones col), so one AV matmul per head also produces the softmax
        # denominator in its 33rd output row.
        vT = big.tile([P, 132 * 32], bf16)
        vT3 = vT.rearrange("p (j h c) -> p j h c", h=4, c=33)
        nc.vector.memset(vT3[:, :, :, 32:33], 1.0)
        with tc.tile_pool(name="psV", bufs=2, space="PSUM") as psV:
            for grp in range(8):
                ps = psV.tile([P, 512], f32)
                for t in range(4):
                    nblk = 4 * grp + t
                    nc.tensor.matmul(
                        ps[:, 128 * t:128 * (t + 1)],
                        lhsT=cb_bf[:, 128 * nblk:128 * (nblk + 1)],
                        rhs=wvT_bf[:],
                        start=True, stop=True, skip_group_check=True)
                nc.vector.tensor_copy(
                    vT3[:, 4 * grp:4 * (grp + 1), :, 0:32],
                    ps.rearrange("p (t h c) -> p t h c", h=4, c=32))

        # ---- q/k projections + L2 norms over full rows
        q_bf = big.tile([P, IPC], bf16)
        k_bf = big.tile([P, N], bf16)
        with tc.tile_pool(name="psP", bufs=2, space="PSUM") as psP, \
             tc.tile_pool(name="sq", bufs=2) as sqp, \
             tc.tile_pool(name="raw", bufs=1) as rawp:
            for which in ("q", "k"):
                w = wqT_bf if which == "q" else wkT_bf
                src = xb_bf if which == "q" else cb_bf
                bias = bq if which == "q" else bk
                if which == "q":
                    keep = rawp.tile([P, IPC], f32, tag="qraw", name="qraw")
                else:
                    keep = rawp.tile([P, N], f32, tag="kraw", name="kraw")
                accs = []
                for chunk in range(2):
                    ps = psP.tile([P, 2048], f32)
                    for t in range(4):
                        c0 = 2048 * chunk + 512 * t
                        nc.tensor.matmul(
                            ps[:, 512 * t:512 * (t + 1)],
                            lhsT=w[:],
                            rhs=src[:, c0:c0 + 512],
                            start=True, stop=True, skip_group_check=True)
                    sq = sqp.tile([P, 2048], f32)
                    acc = stat.tile([P, 1], f32, tag=f"{which}acc{chunk}")
                    nc.scalar.activation(sq[:], ps[:], AF.Square,
                                         bias=bias[:], scale=1.0,
                                         accum_out=acc[:])
                    accs.append(acc)
                    # materialize (bias-added) slice we keep
                    if which == "q":
                        if chunk == 0:
                            nc.vector.tensor_scalar_add(
                                keep[:], ps[:, 0:IPC], bias[:])
                    else:
                        nc.vector.tensor_scalar_add(
                            keep[:, 2048 * chunk:2048 * (chunk + 1)],
                            ps[:], bias[:])
                n2 = stat.tile([P, 1], f32, tag=f"{which}n2")
                nc.vector.tensor_add(n2[:], accs[0][:], accs[1][:])
                nc.vector.tensor_scalar_max(n2[:], n2[:], EPS2)
                lnv = stat.tile([P, 1], f32, tag=f"{which}ln")
                nc.scalar.activation(lnv[:], n2[:], AF.Ln)
                rn = stat.tile([P, 1], f32, tag=f"{which}rn")
                nc.scalar.activation(rn[:], lnv[:], AF.Exp, scale=-0.5)
                if which == "q":
                    nc.vector.tensor_scalar_mul(q_bf[:], keep[:], rn[:])
                else:
                    nc.vector.tensor_scalar_mul(k_bf[:], keep[:], rn[:])

        # ---- stage per-head k/q at base partition 0 (SBUF->SBUF DMA moves
        # across partitions). All sim matmuls then share row group 0, so
        # they serialize on the PE and never write one PSUM (partition,
        # bank) concurrently (concurrent row-tiled matmuls to the same
        # bank are fatal on HW).
        k_h4 = big.tile([32, 4 * N], bf16)
        q_h4 = big.tile([32, 4 * IPC], bf16)
        for h in range(4):
            nc.sync.dma_start(k_h4[:, N * h:N * (h + 1)],
                              k_bf[32 * h:32 * (h + 1), :])
            nc.sync.dma_start(q_h4[:, IPC * h:IPC * (h + 1)],
                              q_bf[32 * h:32 * (h + 1), :])

        # ---- attention
        with tc.tile_pool(name="psS", bufs=3, space="PSUM") as psS, \
             tc.tile_pool(name="psAV", bufs=1, space="PSUM") as psAV, \
             tc.tile_pool(name="pT", bufs=6) as pTp, \
             tc.tile_pool(name="post", bufs=2) as post:
            avs = {}
            for ihalf in range(2):
                avs[ihalf] = [psAV.tile([97, 512], f32, name=f"av{ihalf}_{p}",
                                        tag=f"avp{p}") for p in range(2)]

            def emit_av(rnd, pt):
                ihalf, jblk, iq = rnd
                first = (jblk == 0 and iq == 0)
                last = (jblk == 31 and iq == 1)
                for h in range(4):
                    pair, off = divmod(h, 2)
                    nc.tensor.matmul(
                        avs[ihalf][pair][64 * off:64 * off + 33,
                                         256 * iq:256 * (iq + 1)],
                        lhsT=vT3[:, jblk, h, :],
                        rhs=pt[:, 256 * h:256 * (h + 1)],
                        start=first, stop=last,
                        tile_position=(0, 64 * off),
                        skip_group_check=True)

            pending = []  # (round, pt) awaiting AV emission
            for ihalf in range(2):
                for jblk in range(32):
                    sims = {}
                    for iq in range(2):
                        sims[iq] = psS.tile([P, 1024], f32, name="simps",
                                            tag="simps")
                    for h in range(4):
                        for iq in range(2):
                            i0 = IPC * h + 512 * ihalf + 256 * iq
                            nc.tensor.matmul(
                                sims[iq][:, 256 * h:256 * (h + 1)],
                                lhsT=k_h4[:, N * h + 128 * jblk:
                                          N * h + 128 * (jblk + 1)],
                                rhs=q_h4[:, i0:i0 + 256],
                                start=True, stop=True,
                                skip_group_check=True)
                    while pending:
                        emit_av(*pending.pop(0))
                    for iq in range(2):
                        pt = pTp.tile([P, 1024], bf16, name="pt", tag="pt")
                        nc.scalar.activation(pt[:], sims[iq][:], AF.Exp,
                                             scale=SCALE)
                        pending.append(((ihalf, jblk, iq), pt))
            while pending:
                emit_av(*pending.pop(0))

            for ihalf in range(2):
                # ---- epilogue: rows 0..32 of av_h = sum_j p*v; row 32 =
                # denominator. Reassemble heads to their channel partitions
                # via SBUF->SBUF DMA, divide, project.
                avsb = []
                for pair in range(2):
                    t = post.tile([97, 512], f32, name=f"avsb{pair}",
                                  tag=f"avsb{pair}")
                    nc.vector.tensor_copy(t[0:33, :],
                                          avs[ihalf][pair][0:33, :])
                    nc.vector.tensor_copy(t[64:97, :],
                                          avs[ihalf][pair][64:97, :])
                    avsb.append(t)
                asm = post.tile([P, 512], f32, name="asm")
                den4 = post.tile([4, 512], f32, name="den4")
                for h in range(4):
                    pair, off = divmod(h, 2)
                    nc.sync.dma_start(
                        asm[32 * h:32 * (h + 1), :],
                        avsb[pair][64 * off:64 * off + 32, :])
                    nc.sync.dma_start(
                        den4[h:h + 1, :],
                        avsb[pair][64 * off + 32:64 * off + 33, :])
                rden4 = post.tile([4, 512], f32, name="rden4")
                nc.vector.reciprocal(rden4[:], den4[:])
                rsp = post.tile([P, 512], f32, name="rsp")
                for h in range(4):
                    nc.sync.dma_start(rsp[32 * h:32 * h + 1, :],
                                      rden4[h:h + 1, :])
                bc = psS.tile([P, 512], f32, name="bc", tag="simps")
                for h in range(4):
                    nc.tensor.matmul(
                        bc[32 * h:32 * (h + 1), :],
                        lhsT=ones_sp[32 * h:32 * h + 1, :],
                        rhs=rsp[32 * h:32 * h + 1, :],
                        start=True, stop=True,
                        tile_position=(32 * h, 32 * h),
                        skip_group_check=True)
                rbc = post.tile([P, 512], f32, name="rbc")
                nc.vector.tensor_copy(rbc[:], bc[:])
                outpre = post.tile([P, 512], bf16, name="outpre")
                nc.vector.tensor_mul(outpre[:], asm[:], rbc[:])
                po = psS.tile([P, 512], f32, name="po", tag="simps")
                nc.tensor.matmul(po[:], lhsT=woT_bf[:], rhs=outpre[:],
                                 start=True, stop=True)
                y_sb = post.tile([P, 512], f32, name="ysb")
                nc.vector.tensor_scalar_add(y_sb[:], po[:], boe[:])
                nc.sync.dma_start(
                    y_d[:, 512 * ihalf:512 * (ihalf + 1)], y_sb[:])


_NC_CACHE = None


def _get_program():
    global _NC_CACHE
    if _NC_CACHE is None:
        _NC_CACHE = _build_program()
    return _NC_CACHE


def kernel(**inputs):
    global LAST_RESULTS
    f = lambda k: np.ascontiguousarray(np.asarray(inputs[k], dtype=np.float32))
    x, cond = f("x"), f("cond_x")
    Wq, Wk, Wv, Wo = f("Wq"), f("Wk"), f("Wv"), f("Wo")
    bq, bk, bv, bo = f("bq"), f("bk"), f("bv"), f("bo")

    B = x.shape[0]
    xf = x.reshape(B, P, N)
    cf = cond.reshape(B, P, N)
    bo_eff = bo + Wo @ bv  # bv commutes through the attention average

    wqT = np.ascontiguousarray(Wq.T)
    wkT = np.ascontiguousarray(Wk.T)
    wvT = np.ascontiguousarray(Wv.T)
    woT = np.ascontiguousarray(Wo.T)

    in_maps = []
    for core in range(NCORES):
        b, q4 = divmod(core, 4)
        i0 = 1024 * q4
        in_maps.append({
            "xb": np.ascontiguousarray(np.roll(xf[b], -i0, axis=1)),
            "cb": np.ascontiguousarray(cf[b]),
            "wqT": wqT, "wkT": wkT, "wvT": wvT, "woT": woT,
            "bq": bq.reshape(P, 1), "bk": bk.reshape(P, 1),
            "boe": bo_eff.reshape(P, 1),
        })

    nc = _get_program()
    res = bass_utils.run_bass_kernel_spmd(
        nc, in_maps, core_ids=list(range(NCORES)))
    LAST_RESULTS = res

    out = np.empty((B, P, N), np.float32)
    for core in range(NCORES):
        b, q4 = divmod(core, 4)
        out[b, :, 1024 * q4:1024 * (q4 + 1)] = res.results[core]["y"]
    return out.reshape(B, P, 16, 16, 16)


if __name__ == "__main__":
    rng = np.random.default_rng(0)
    ins = {
        "x": rng.standard_normal((2, P, 16, 16, 16), dtype=np.float32),
        "cond_x": rng.standard_normal((2, P, 16, 16, 16), dtype=np.float32),
    }
    for nm in ("q", "k", "v", "o"):
        ins[f"W{nm}"] = rng.standard_normal((P, P), dtype=np.float32) / np.sqrt(P)
        ins[f"b{nm}"] = rng.standard_normal((P,), dtype=np.float32) * 0.01
    out = kernel(**ins)
    print("kernel ran, out shape", out.shape)

